# revision 10
# baseline (speedup 1.0000x reference)
"""GNN message-passing (CPF/PLP) Bass kernel for 8 trn2 NeuronCores.

Gather-free design: nodes are sharded into eighths (12500 per core); each
core handles BOTH graphs for its node range, so no collectives are needed.
The host presorts each core's edges by destination rank into a quantized
column grid, so edge-softmax + segment-sum lower to static strided
tensor_tensor/tensor_reduce ops. Per-edge h[src] message payloads are laid
out by the host (bf16): layer 1 uses label_init directly; between the two
launches the host assembles h1 and lays out layer-2 messages. Launch 2 also
runs the attention mix and the feature MLP (TensorE) and emits the final
output.
"""

import numpy as np

N, C, G, L, E, F, H = 100000, 16, 2, 2, 3200000, 512, 64
P = 128
S8 = 12500               # real nodes per core
ROWS = 98
SLAB = P * ROWS          # 12544
CT = 128                 # compute tile columns
MLPB = 384               # mlp block columns (3 rows)

_CACHE = {}


# ---------------------------------------------------------------------------
# host preprocessing
# ---------------------------------------------------------------------------

def _row_quant(cnt_rank):
    g = cnt_rank.reshape(ROWS, P).max(axis=1)
    return ((g + 1) // 2) * 2


def _grid_from_g(g):
    assert g.max() <= CT
    offs = np.zeros(ROWS, np.int64)
    pos = 0
    for k in range(ROWS):
        gk = int(g[k])
        if gk == 0:
            offs[k] = pos
            continue
        if (pos % CT) + gk > CT:
            pos = ((pos // CT) + 1) * CT
        offs[k] = pos
        pos += gk
    K = ((pos + CT - 1) // CT) * CT
    tiles = []
    for t in range(K // CT):
        lo, hi = t * CT, (t + 1) * CT
        ks = [k for k in range(ROWS) if g[k] > 0 and lo <= offs[k] < hi]
        runs = []
        i = 0
        while i < len(ks):
            j = i
            while (j + 1 < len(ks) and g[ks[j + 1]] == g[ks[i]]
                   and offs[ks[j + 1]] == offs[ks[j]] + g[ks[j]]):
                j += 1
            runs.append((ks[i], j - i + 1, int(g[ks[i]]),
                         int(offs[ks[i]]) - lo))
            i = j + 1
        tiles.append(runs)
    return offs, K, tiles


def _edge_slots(dst_rank, offs):
    """Edges given by dst rank (len nE). Returns (p, col) slot per edge."""
    order = np.argsort(dst_rank, kind="stable")
    r_s = dst_rank[order]
    seg_start = np.r_[True, r_s[1:] != r_s[:-1]]
    run_first = np.nonzero(seg_start)[0]
    run_id = np.cumsum(seg_start) - 1
    j = np.arange(len(r_s)) - run_first[run_id]
    p = r_s % P
    col = offs[r_s // P] + j
    inv = np.empty_like(order)
    inv[order] = np.arange(len(order))
    return p[inv], col[inv]


def _host_prep(inputs):
    src = np.asarray(inputs["src"])
    dst = np.asarray(inputs["dst"])
    e_edge = np.asarray(inputs["e_edge"]).astype(np.float32)
    label_init = np.asarray(inputs["label_init"]).astype(np.float32)
    labels_one_hot = np.asarray(inputs["labels_one_hot"]).astype(np.float32)
    train_mask = np.asarray(inputs["train_mask"]).astype(np.float32)
    attention = np.asarray(inputs["attention"]).astype(np.float32)
    alpha = np.asarray(inputs["alpha"]).astype(np.float32)
    features = np.asarray(inputs["features"]).astype(np.float32)

    pr = {"edges": [], "l1": [], "l2": [], "masks": [], "final": []}
    deg = np.zeros((G, 8, SLAB), np.int64)
    esel = [[None] * G for _ in range(8)]
    vloc = [[None] * G for _ in range(8)]
    for q in range(8):
        for g in range(G):
            sel = np.nonzero((dst[g] >= S8 * q) & (dst[g] < S8 * (q + 1)))[0]
            esel[q][g] = sel
            vloc[q][g] = dst[g][sel] - S8 * q
            np.add.at(deg[g, q], (vloc[q][g],), 1)

    # L1: per (q, g) rank order by own degree; grid SHARED across cores
    # (SPMD: all cores run the same program) -> row-quant max over q.
    rank1 = np.zeros((8, G, SLAB), np.int64)
    g1rows = np.zeros((G, 8, ROWS), np.int64)
    for q in range(8):
        for g in range(G):
            o = np.argsort(-deg[g, q], kind="stable")
            rk = np.empty(SLAB, np.int64)
            rk[o] = np.arange(SLAB)
            rank1[q, g] = rk
            g1rows[g, q] = _row_quant(deg[g, q][o])
    grids1 = [_grid_from_g(g1rows[g].max(axis=0)) for g in range(G)]

    # L2: per q shared rank order by total degree; grid shared across cores
    rankT = np.zeros((8, SLAB), np.int64)
    orderT = np.zeros((8, SLAB), np.int64)
    g2rows = np.zeros((G, 8, ROWS), np.int64)
    for q in range(8):
        tot = deg[0, q] + deg[1, q]
        o = np.argsort(-tot, kind="stable")
        orderT[q] = o
        rk = np.empty(SLAB, np.int64)
        rk[o] = np.arange(SLAB)
        rankT[q] = rk
        for g in range(G):
            g2rows[g, q] = _row_quant(deg[g, q][o])
    grids2 = [_grid_from_g(g2rows[g].max(axis=0)) for g in range(G)]

    def masks_for(order_rank_inv, q):
        # order_rank_inv: rank -> vloc (len SLAB)
        vg = order_rank_inv + S8 * q
        valid = (order_rank_inv < S8).astype(np.float32)
        m = train_mask[np.minimum(vg, N - 1), 0] * valid
        ml = (1.0 - m) * valid
        moh = labels_one_hot[np.minimum(vg, N - 1)] * m[:, None]
        mlT = ml.reshape(ROWS, P).T.copy()
        mohT = moh.reshape(ROWS, P, C).transpose(1, 0, 2).copy()
        return mlT, mohT

    in1, in2 = [], []
    meta1, meta2 = [], []
    eslots = {}
    for q in range(8):
        m1, m2_ = {}, {}
        meta1q, meta2q = [], []
        for g in range(G):
            sel, vl = esel[q][g], vloc[q][g]
            u_src = src[g][sel]
            # ---- L1 ----
            offs, K, tiles = grids1[g]
            rk = rank1[q, g][vl]
            p_, col = _edge_slots(rk, offs)
            msg = np.zeros((P, K, C + 1), np.float32)
            msg[p_, col, 0:C] = label_init[u_src]
            msg[p_, col, C] = 1.0
            ee = np.full((P, K), -1e30, np.float32)
            ee[p_, col] = e_edge[0, g][sel]
            m1[f"msg{g}"] = msg
            m1[f"ee{g}"] = ee
            o1 = np.empty(SLAB, np.int64)
            o1[rank1[q, g]] = np.arange(SLAB)  # rank -> vloc
            ml_, moh_ = masks_for(o1, q)
            m1[f"ml{g}"] = ml_
            m1[f"moh{g}"] = moh_
            meta1q.append((K, tiles))
            # ---- L2 ----
            offs, K2, tiles2 = grids2[g]
            rk2 = rankT[q][vl]
            p2, col2 = _edge_slots(rk2, offs)
            eslots.setdefault(q, {})[g] = (sel, p2, col2, K2)
            ee2 = np.full((P, K2), -1e30, np.float32)
            ee2[p2, col2] = e_edge[1, g][sel]
            m2_[f"ee{g}"] = ee2
            mlT2, mohT2 = masks_for(orderT[q], q)
            m2_[f"ml{g}"] = mlT2
            m2_[f"moh{g}"] = mohT2
            meta2q.append((K2, tiles2))
        # final-stage tensors in rankT order
        vg = orderT[q] + S8 * q
        vgc = np.minimum(vg, N - 1)
        validn = (orderT[q] < S8).astype(np.float32)
        att = attention[vgc, :, 0] * validn[:, None]
        m2_["attf"] = att.reshape(ROWS, P, G).transpose(1, 0, 2).copy()
        m2_["alphaf"] = (alpha[vgc, 0] * validn).reshape(ROWS, P).T.copy()
        featv = features[vgc] * validn[:, None]
        m2_["featT"] = featv.T.reshape(4, P, SLAB).copy()
        m2_["w1"] = np.asarray(inputs["w1"]).astype(np.float32)
        m2_["b1t"] = np.asarray(inputs["b1"]).astype(np.float32).reshape(H, 1)
        m2_["w2"] = np.asarray(inputs["w2"]).astype(np.float32)
        m2_["b2t"] = np.asarray(inputs["b2"]).astype(np.float32).reshape(C, 1)
        m2_["ident"] = np.eye(C, dtype=np.float32)
        in1.append(m1)
        in2.append(m2_)
        meta1.append(meta1q)
        meta2.append(meta2q)

    pr = {"in1": in1, "in2": in2, "meta1": meta1, "meta2": meta2,
          "rank1": rank1, "rankT": rankT, "orderT": orderT,
          "eslots": eslots}
    return pr


def _fill_l2_msgs(pr, h1full):
    """After launch 1: build layer-2 message payloads from assembled h1."""
    for q in range(8):
        m2_ = pr["in2"][q]
        for g in range(G):
            sel, p2, col2, K2 = pr["eslots"][q][g]
            src = pr["_src"]
            u_src = src[g][sel]
            msg = np.zeros((P, K2, C + 1), np.float32)
            msg[p2, col2, 0:C] = h1full[g][u_src]
            msg[p2, col2, C] = 1.0
            m2_[f"msg{g}"] = msg


def _patch_tile():
    import concourse.tile as tile
    import concourse.mybir as mybir
    from concourse.vector_clock import ScopedClock

    def _drain_and_barrier(self, tick_clock, wait_clock):
        nc = self.nc
        drain_inst = nc.sync.drain()
        wait_clock.add_sem_waits(
            drain_inst.ins, ScopedClock({None: tick_clock.global_clock}))
        si = drain_inst.ins.sync_info
        if si is not None and len(si.on_wait) > 1:
            waits = list(si.on_wait)
            si.on_wait = waits[:1]
            rest = waits[1:]
            while rest:
                extra = nc.sync.drain()
                chunk, rest = rest[:1], rest[1:]
                esi = extra.ins.sync_info
                if esi is None:
                    extra.ins.sync_info = mybir.SyncInfo(
                        on_wait=chunk, on_update=[])
                else:
                    esi.on_wait = chunk
        nc.all_engine_barrier()
        assert self.sems is not None
        popped = nc._tile_sem_poison_stack.pop()
        assert popped is self._sem_poison
        nc.clear_and_free_semaphores(list(self.sems.allocated().values()))
        nc.all_engine_barrier()

    tile.TileContext._drain_and_barrier = _drain_and_barrier


def _split_excess_waits(nc, limit=1):
    import concourse.mybir as mybir
    seen, bbs = set(), []
    for name, bbc in nc.bb_map.items():
        bb = bbc.bb if hasattr(bbc, "bb") else bbc
        if id(bb) not in seen:
            seen.add(id(bb))
            bbs.append(bb)
    cur = nc.cur_bb.bb
    for bb in bbs:
        insts = bb.instructions
        out, changed = [], False
        for inst in insts:
            si = inst.sync_info
            if si is not None and len(si.on_wait) > limit:
                waits = list(si.on_wait)
                keep, extra = waits[:limit], waits[limit:]
                for w in extra:
                    nop = nc.engines[inst.engine].nop().ins
                    cl = cur.instructions
                    assert cl and cl[-1].name == nop.name
                    cur.instructions = cl[:-1]
                    nop.sync_info = mybir.SyncInfo(on_wait=[w], on_update=[])
                    out.append(nop)
                si.on_wait = keep
                changed = True
            out.append(inst)
        if changed:
            bb.instructions = out



# ---------------------------------------------------------------------------
# device programs
# ---------------------------------------------------------------------------

def _layer_block(nc, tc, mb, pools, ext, meta_q, tag_sfx=""):
    """Emit per-graph edge-softmax + segment-sum for one launch.
    Returns list of h tiles (one per graph), each [P, ROWS, C] f32."""
    dt = mb.dt
    msgp, epool, accp = pools
    hs = []
    for g in range(G):
        K, tiles = meta_q[g]
        u = accp.tile([P, ROWS, C + 1], dt.float32, name=f"u{g}{tag_sfx}",
                      tag=f"u{g}")
        nc.vector.memset(u[:], 0.0)
        for t in range(K // CT):
            msg = msgp.tile([P, CT, C + 1], dt.float32,
                            name=f"m{g}{t}{tag_sfx}", tag="msg")
            nc.sync.dma_start(
                out=msg[:], in_=ext[f"msg{g}"][:, t * CT:(t + 1) * CT, :])
            et = epool.tile([P, CT], dt.float32, name=f"e{g}{t}{tag_sfx}",
                            tag="et")
            nc.sync.dma_start(out=et[:],
                              in_=ext[f"ee{g}"][:, t * CT:(t + 1) * CT])
            ex = epool.tile([P, CT], dt.float32, name=f"x{g}{t}{tag_sfx}",
                            tag="ex")
            nc.scalar.activation(ex[:], et[:],
                                 mb.ActivationFunctionType.Exp)
            prod = msgp.tile([P, CT, C + 1], dt.float32,
                             name=f"p{g}{t}{tag_sfx}", tag="prod")
            nc.vector.tensor_tensor(
                out=prod[:], in0=msg[:],
                in1=ex[:].to_broadcast([P, CT, C + 1]),
                op=mb.AluOpType.mult)
            for (k0, nk, g_, off) in tiles[t]:
                inap = prod[:, off:off + nk * g_, :].rearrange(
                    "p (nk g) c -> p nk c g", g=g_)
                nc.vector.tensor_reduce(
                    out=u[:, k0:k0 + nk, :], in_=inap,
                    axis=mb.AxisListType.X, op=mb.AluOpType.add)
        ml = accp.tile([P, ROWS], dt.float32, name=f"ml{g}{tag_sfx}",
                       tag=f"ml{g}")
        nc.sync.dma_start(out=ml[:], in_=ext[f"ml{g}"][:])
        moh = accp.tile([P, ROWS, C], dt.float32, name=f"moh{g}{tag_sfx}",
                        tag=f"moh{g}")
        nc.sync.dma_start(out=moh[:], in_=ext[f"moh{g}"][:])
        s = u[:, :, C]
        nc.vector.tensor_scalar_max(s, s, 1.0)
        rec = accp.tile([P, ROWS], dt.float32, name=f"rc{g}{tag_sfx}",
                        tag=f"rec{g}")
        nc.vector.reciprocal(out=rec[:], in_=s)
        h = accp.tile([P, ROWS, C], dt.float32, name=f"h{g}{tag_sfx}",
                      tag=f"h{g}")
        nc.vector.tensor_tensor(
            out=h[:], in0=u[:, :, 0:C],
            in1=rec[:].to_broadcast([P, ROWS, C]), op=mb.AluOpType.mult)
        nc.vector.tensor_tensor(
            out=h[:], in0=h[:], in1=ml[:].to_broadcast([P, ROWS, C]),
            op=mb.AluOpType.mult)
        nc.vector.tensor_tensor(out=h[:], in0=h[:], in1=moh[:],
                                op=mb.AluOpType.add)
        hs.append(h)
    return hs


def _declare_layer_inputs(nc, dt, meta_q):
    ext = {}
    for g in range(G):
        K, _ = meta_q[g]
        ext[f"msg{g}"] = nc.declare_dram_parameter(
            f"msg{g}", [P, K, C + 1], dt.float32, isOutput=False)
        ext[f"ee{g}"] = nc.declare_dram_parameter(
            f"ee{g}", [P, K], dt.float32, isOutput=False)
        ext[f"ml{g}"] = nc.declare_dram_parameter(
            f"ml{g}", [P, ROWS], dt.float32, isOutput=False)
        ext[f"moh{g}"] = nc.declare_dram_parameter(
            f"moh{g}", [P, ROWS, C], dt.float32, isOutput=False)
    return ext


def _build_l1(meta_q):
    import concourse.bass as bass
    import concourse.mybir as mb
    from concourse.tile import TileContext

    _patch_tile()
    dt = mb.dt
    nc = bass.Bass("TRN2", target_bir_lowering=False, debug=False)
    ext = _declare_layer_inputs(nc, dt, meta_q)
    outs = [nc.declare_dram_parameter(f"out{g}", [P, ROWS, C], dt.float32,
                                      isOutput=True) for g in range(G)]
    with TileContext(nc) as tc:
        with (
            tc.tile_pool(name="msgp", bufs=3) as msgp,
            tc.tile_pool(name="epool", bufs=3) as epool,
            tc.tile_pool(name="accp", bufs=1) as accp,
        ):
            hs = _layer_block(nc, tc, mb, (msgp, epool, accp), ext, meta_q)
            for g in range(G):
                nc.sync.dma_start(out=outs[g][:], in_=hs[g][:])
    _split_excess_waits(nc)
    mb.codegen_inst_isa_subclasses(nc)
    return nc


def _build_l2(meta_q):
    import concourse.bass as bass
    import concourse.mybir as mb
    from concourse.tile import TileContext

    _patch_tile()
    dt = mb.dt
    nc = bass.Bass("TRN2", target_bir_lowering=False, debug=False)
    ext = _declare_layer_inputs(nc, dt, meta_q)
    for nm, shp in (("attf", [P, ROWS, G]), ("alphaf", [P, ROWS]),
                    ("featT", [4, P, SLAB]), ("w1", [F, H]),
                    ("b1t", [H, 1]), ("w2", [H, C]), ("b2t", [C, 1]),
                    ("ident", [C, C])):
        ext[nm] = nc.declare_dram_parameter(nm, shp, dt.float32,
                                            isOutput=False)
    out_ext = nc.declare_dram_parameter("out", [P, ROWS, C], dt.float32,
                                        isOutput=True)
    with TileContext(nc) as tc:
        with (
            tc.tile_pool(name="msgp", bufs=3) as msgp,
            tc.tile_pool(name="epool", bufs=3) as epool,
            tc.tile_pool(name="accp", bufs=1) as accp,
            tc.tile_pool(name="wkf", bufs=2) as wkf,
            tc.tile_pool(name="psp", bufs=2, space="PSUM") as psp,
        ):
            hs = _layer_block(nc, tc, mb, (msgp, epool, accp), ext, meta_q)

            # attention softmax + logits
            att = accp.tile([P, ROWS, G], dt.float32, name="atts",
                            tag="atts")
            nc.sync.dma_start(out=att[:], in_=ext["attf"][:])
            ea = wkf.tile([P, ROWS, G], dt.float32, name="ea", tag="ea")
            nc.scalar.activation(ea[:], att[:],
                                 mb.ActivationFunctionType.Exp)
            easum = wkf.tile([P, ROWS], dt.float32, name="easum",
                             tag="easum")
            nc.vector.tensor_reduce(out=easum[:], in_=ea[:],
                                    axis=mb.AxisListType.X,
                                    op=mb.AluOpType.add)
            erec = wkf.tile([P, ROWS], dt.float32, name="erec", tag="easum")
            nc.vector.reciprocal(out=erec[:], in_=easum[:])
            logits = accp.tile([P, ROWS, C], dt.float32, name="logits",
                               tag="logits")
            t0 = wkf.tile([P, ROWS, C], dt.float32, name="t0", tag="t0")
            nc.vector.tensor_tensor(
                out=logits[:], in0=hs[0][:],
                in1=ea[:, :, 0].to_broadcast([P, ROWS, C]),
                op=mb.AluOpType.mult)
            nc.vector.tensor_tensor(
                out=t0[:], in0=hs[1][:],
                in1=ea[:, :, 1].to_broadcast([P, ROWS, C]),
                op=mb.AluOpType.mult)
            nc.vector.tensor_tensor(out=logits[:], in0=logits[:],
                                    in1=t0[:], op=mb.AluOpType.add)
            nc.vector.tensor_tensor(
                out=logits[:], in0=logits[:],
                in1=erec[:].to_broadcast([P, ROWS, C]),
                op=mb.AluOpType.mult)

            # MLP over slab nodes
            w1s = accp.tile([P, 4, H], dt.float32, name="w1s", tag="w1s")
            nc.sync.dma_start(out=w1s[:], in_=ext["w1"][:].rearrange(
                "(c p) h -> p c h", c=4))
            w2s = accp.tile([H, C], dt.float32, name="w2s", tag="w2s")
            nc.sync.dma_start(out=w2s[:], in_=ext["w2"][:])
            b1s = accp.tile([H, 1], dt.float32, name="b1s", tag="b1s")
            nc.sync.dma_start(out=b1s[:], in_=ext["b1t"][:])
            b2s = accp.tile([C, 1], dt.float32, name="b2s", tag="b2s")
            nc.sync.dma_start(out=b2s[:], in_=ext["b2t"][:])
            idn = accp.tile([C, C], dt.float32, name="idn", tag="idn")
            nc.sync.dma_start(out=idn[:], in_=ext["ident"][:])

            mlpn = accp.tile([P, ROWS, C], dt.float32, name="mlpn",
                             tag="mlpn")
            nblk = SLAB // MLPB  # 32 full blocks
            blocks = [(b * MLPB, MLPB) for b in range(nblk)]
            if SLAB % MLPB:
                blocks.append((nblk * MLPB, SLAB % MLPB))
            for bi, (c0, ncols) in enumerate(blocks):
                ps1 = psp.tile([H, ncols], dt.float32, name=f"ps1{bi}",
                               tag="ps1")
                for j in range(4):
                    xt = wkf.tile([P, ncols], dt.float32, name=f"xt{bi}{j}",
                                  tag="xt")
                    nc.sync.dma_start(
                        out=xt[:], in_=ext["featT"][j, :, c0:c0 + ncols])
                    nc.tensor.matmul(out=ps1[:], lhsT=w1s[:, j, :],
                                     rhs=xt[:], start=(j == 0),
                                     stop=(j == 3))
                r1 = wkf.tile([H, ncols], dt.float32, name=f"r1{bi}",
                              tag="r1")
                nc.scalar.activation(r1[:], ps1[:],
                                     mb.ActivationFunctionType.Relu,
                                     bias=b1s[:])
                ps2 = psp.tile([C, ncols], dt.float32, name=f"ps2{bi}",
                               tag="ps2")
                nc.tensor.matmul(out=ps2[:], lhsT=w2s[:], rhs=r1[:],
                                 start=True, stop=True)
                m2 = wkf.tile([C, ncols], dt.float32, name=f"m2{bi}",
                              tag="m2")
                nc.vector.tensor_scalar_add(m2[:], ps2[:], b2s[:])
                for cch in range(ncols // P):
                    pst = psp.tile([P, C], dt.float32,
                                   name=f"pst{bi}{cch}", tag="pst")
                    nc.tensor.transpose(out=pst[:],
                                        in_=m2[:, cch * P:(cch + 1) * P],
                                        identity=idn[:])
                    nc.vector.tensor_copy(
                        out=mlpn[:, c0 // P + cch, :], in_=pst[:])

            alp = accp.tile([P, ROWS], dt.float32, name="alp", tag="alp")
            nc.sync.dma_start(out=alp[:], in_=ext["alphaf"][:])
            sgp = wkf.tile([P, ROWS], dt.float32, name="sgp", tag="sgp")
            nc.scalar.activation(sgp[:], alp[:],
                                 mb.ActivationFunctionType.Sigmoid)
            sgn = wkf.tile([P, ROWS], dt.float32, name="sgn", tag="sgn")
            nc.scalar.activation(sgn[:], alp[:],
                                 mb.ActivationFunctionType.Sigmoid,
                                 scale=-1.0)
            fout = accp.tile([P, ROWS, C], dt.float32, name="fout",
                             tag="fout")
            nc.vector.tensor_tensor(
                out=fout[:], in0=logits[:],
                in1=sgp[:].to_broadcast([P, ROWS, C]),
                op=mb.AluOpType.mult)
            t1 = wkf.tile([P, ROWS, C], dt.float32, name="t1", tag="t0")
            nc.vector.tensor_tensor(
                out=t1[:], in0=mlpn[:],
                in1=sgn[:].to_broadcast([P, ROWS, C]),
                op=mb.AluOpType.mult)
            nc.vector.tensor_tensor(out=fout[:], in0=fout[:], in1=t1[:],
                                    op=mb.AluOpType.add)
            nc.sync.dma_start(out=out_ext[:], in_=fout[:])
    _split_excess_waits(nc)
    mb.codegen_inst_isa_subclasses(nc)
    return nc


def _kernel_host(**inputs):
    """Exact reference semantics in numpy (f32)."""
    src = np.asarray(inputs["src"]); dst = np.asarray(inputs["dst"])
    e_edge = np.asarray(inputs["e_edge"], dtype=np.float32)
    label_init = np.asarray(inputs["label_init"], dtype=np.float32)
    labels_one_hot = np.asarray(inputs["labels_one_hot"], dtype=np.float32)
    alpha = np.asarray(inputs["alpha"], dtype=np.float32)
    attention = np.asarray(inputs["attention"], dtype=np.float32)
    w1 = np.asarray(inputs["w1"], dtype=np.float32)
    b1 = np.asarray(inputs["b1"], dtype=np.float32)
    w2 = np.asarray(inputs["w2"], dtype=np.float32)
    b2 = np.asarray(inputs["b2"], dtype=np.float32)
    train_mask = np.asarray(inputs["train_mask"])
    mask = train_mask.astype(np.float32)
    masked_label = 1.0 - mask
    masked_one_hot = labels_one_hot * mask
    h_list = []
    for g in range(G):
        h = label_init
        d = dst[g]; s_ = src[g]
        for l in range(L):
            e = e_edge[l, g]
            m = np.full(N, -np.inf, np.float32)
            np.maximum.at(m, d, e)
            ex = np.exp(e - m[d])
            ssum = np.zeros(N, np.float32)
            np.add.at(ssum, d, ex)
            a = ex / ssum[d]
            hn = np.zeros((N, C), np.float32)
            np.add.at(hn, d, h[s_] * a[:, None])
            h = hn * masked_label + masked_one_hot
        h_list.append(h)
    x = np.stack(h_list, axis=-1)                      # [N, C, G]
    att = attention[..., 0]                            # [N, G]
    att = att - att.max(axis=1, keepdims=True)
    ea = np.exp(att)
    attn = ea / ea.sum(axis=1, keepdims=True)
    logits = np.einsum("ncg,ng->nc", x, attn)
    mlp = np.maximum(features_mm(inputs, w1) + b1, 0.0) @ w2 + b2
    sa = 1.0 / (1.0 + np.exp(-alpha))
    return (sa * logits + (1.0 - sa) * mlp).astype(np.float32)


def features_mm(inputs, w1):
    f = np.asarray(inputs["features"], dtype=np.float32)
    return f @ w1



def kernel(**inputs):
    import os
    if os.environ.get("GNN_HOST") == "1":
        return _kernel_host(**inputs)
    import time
    from concourse.bass_utils import run_bass_kernel_spmd

    t0 = time.perf_counter()
    pr = _host_prep(inputs)
    pr["_src"] = np.asarray(inputs["src"])

    meta1 = pr["meta1"]
    meta2 = pr["meta2"]

    key1 = "l1" + str(meta1[0])
    if key1 not in _CACHE:
        _CACHE[key1] = _build_l1(meta1[0])
    nc1 = _CACHE[key1]
    t1 = time.perf_counter()
    res1 = run_bass_kernel_spmd(nc1, pr["in1"], list(range(8)))
    _CACHE["res1"] = res1
    t2 = time.perf_counter()

    h1full = [np.zeros((N, C), np.float32) for _ in range(G)]
    for q in range(8):
        for g in range(G):
            hq = res1.results[q][f"out{g}"]          # [P, ROWS, C] by rank
            nat = np.asarray(hq).transpose(1, 0, 2).reshape(SLAB, C)
            h1full[g][S8 * q:S8 * (q + 1)] = nat[pr["rank1"][q, g][:S8]]
    _fill_l2_msgs(pr, h1full)
    t3 = time.perf_counter()

    key2 = "l2" + str(meta2[0])
    if key2 not in _CACHE:
        _CACHE[key2] = _build_l2(meta2[0])
    nc2 = _CACHE[key2]
    t4 = time.perf_counter()
    res2 = run_bass_kernel_spmd(nc2, pr["in2"], list(range(8)))
    _CACHE["res"] = res2
    t5 = time.perf_counter()
    import sys
    print(f"[kernel] prep {t1-t0:.2f}s run1 {t2-t1:.2f}s fill {t3-t2:.2f}s "
          f"build2 {t4-t3:.2f}s run2 {t5-t4:.2f}s", file=sys.stderr)

    out = np.zeros((N, C), np.float32)
    for q in range(8):
        oq = np.asarray(res2.results[q]["out"]).transpose(
            1, 0, 2).reshape(SLAB, C)
        out[S8 * q:S8 * (q + 1)] = oq[pr["rankT"][q][:S8]]
    return out


# revision 11
# speedup vs baseline: 1.9150x; 1.9150x over previous
"""GNN message-passing (CPF/PLP) Bass kernel for 8 trn2 NeuronCores.

Gather-free design: nodes are sharded into eighths (12500 per core); each
core handles BOTH graphs for its node range, so no collectives are needed.
The host presorts each core's edges by destination rank into a quantized
column grid, so edge-softmax + segment-sum lower to static strided
tensor_tensor/tensor_reduce ops. Per-edge h[src] message payloads are laid
out by the host (bf16): layer 1 uses label_init directly; between the two
launches the host assembles h1 and lays out layer-2 messages. Launch 2 also
runs the attention mix and the feature MLP (TensorE) and emits the final
output.
"""

import numpy as np
from ml_dtypes import bfloat16

N, C, G, L, E, F, H = 100000, 16, 2, 2, 3200000, 512, 64
P = 128
S8 = 12500               # real nodes per core
ROWS = 98
SLAB = P * ROWS          # 12544
CT = 128                 # compute tile columns
MLPB = 384               # mlp block columns (3 rows)

_CACHE = {}


# ---------------------------------------------------------------------------
# host preprocessing
# ---------------------------------------------------------------------------

def _row_quant(cnt_rank):
    g = cnt_rank.reshape(ROWS, P).max(axis=1)
    return ((g + 1) // 2) * 2


def _grid_from_g(g):
    assert g.max() <= CT
    offs = np.zeros(ROWS, np.int64)
    pos = 0
    for k in range(ROWS):
        gk = int(g[k])
        if gk == 0:
            offs[k] = pos
            continue
        if (pos % CT) + gk > CT:
            pos = ((pos // CT) + 1) * CT
        offs[k] = pos
        pos += gk
    K = ((pos + CT - 1) // CT) * CT
    tiles = []
    for t in range(K // CT):
        lo, hi = t * CT, (t + 1) * CT
        ks = [k for k in range(ROWS) if g[k] > 0 and lo <= offs[k] < hi]
        runs = []
        i = 0
        while i < len(ks):
            j = i
            while (j + 1 < len(ks) and g[ks[j + 1]] == g[ks[i]]
                   and offs[ks[j + 1]] == offs[ks[j]] + g[ks[j]]):
                j += 1
            runs.append((ks[i], j - i + 1, int(g[ks[i]]),
                         int(offs[ks[i]]) - lo))
            i = j + 1
        tiles.append(runs)
    return offs, K, tiles


def _edge_slots(dst_rank, offs):
    """Edges given by dst rank (len nE). Returns (p, col) slot per edge."""
    order = np.argsort(dst_rank, kind="stable")
    r_s = dst_rank[order]
    seg_start = np.r_[True, r_s[1:] != r_s[:-1]]
    run_first = np.nonzero(seg_start)[0]
    run_id = np.cumsum(seg_start) - 1
    j = np.arange(len(r_s)) - run_first[run_id]
    p = r_s % P
    col = offs[r_s // P] + j
    inv = np.empty_like(order)
    inv[order] = np.arange(len(order))
    return p[inv], col[inv]


def _host_prep(inputs):
    src = np.asarray(inputs["src"])
    dst = np.asarray(inputs["dst"])
    e_edge = np.asarray(inputs["e_edge"]).astype(np.float32)
    label_init = np.asarray(inputs["label_init"]).astype(np.float32)
    labels_one_hot = np.asarray(inputs["labels_one_hot"]).astype(np.float32)
    train_mask = np.asarray(inputs["train_mask"]).astype(np.float32)
    attention = np.asarray(inputs["attention"]).astype(np.float32)
    alpha = np.asarray(inputs["alpha"]).astype(np.float32)
    features = np.asarray(inputs["features"]).astype(np.float32)

    pr = {"edges": [], "l1": [], "l2": [], "masks": [], "final": []}
    deg = np.zeros((G, 8, SLAB), np.int64)
    esel = [[None] * G for _ in range(8)]
    vloc = [[None] * G for _ in range(8)]
    for q in range(8):
        for g in range(G):
            sel = np.nonzero((dst[g] >= S8 * q) & (dst[g] < S8 * (q + 1)))[0]
            esel[q][g] = sel
            vloc[q][g] = dst[g][sel] - S8 * q
            np.add.at(deg[g, q], (vloc[q][g],), 1)

    # L1: per (q, g) rank order by own degree; grid SHARED across cores
    # (SPMD: all cores run the same program) -> row-quant max over q.
    rank1 = np.zeros((8, G, SLAB), np.int64)
    g1rows = np.zeros((G, 8, ROWS), np.int64)
    for q in range(8):
        for g in range(G):
            o = np.argsort(-deg[g, q], kind="stable")
            rk = np.empty(SLAB, np.int64)
            rk[o] = np.arange(SLAB)
            rank1[q, g] = rk
            g1rows[g, q] = _row_quant(deg[g, q][o])
    grids1 = [_grid_from_g(g1rows[g].max(axis=0)) for g in range(G)]

    # L2: per q shared rank order by total degree; grid shared across cores
    rankT = np.zeros((8, SLAB), np.int64)
    orderT = np.zeros((8, SLAB), np.int64)
    g2rows = np.zeros((G, 8, ROWS), np.int64)
    for q in range(8):
        tot = deg[0, q] + deg[1, q]
        o = np.argsort(-tot, kind="stable")
        orderT[q] = o
        rk = np.empty(SLAB, np.int64)
        rk[o] = np.arange(SLAB)
        rankT[q] = rk
        for g in range(G):
            g2rows[g, q] = _row_quant(deg[g, q][o])
    grids2 = [_grid_from_g(g2rows[g].max(axis=0)) for g in range(G)]

    def masks_for(order_rank_inv, q):
        # order_rank_inv: rank -> vloc (len SLAB)
        vg = order_rank_inv + S8 * q
        valid = (order_rank_inv < S8).astype(np.float32)
        m = train_mask[np.minimum(vg, N - 1), 0] * valid
        ml = (1.0 - m) * valid
        moh = labels_one_hot[np.minimum(vg, N - 1)] * m[:, None]
        mlT = ml.reshape(ROWS, P).T.copy()
        mohT = moh.reshape(ROWS, P, C).transpose(1, 0, 2).copy()
        return mlT, mohT

    in1, in2 = [], []
    meta1, meta2 = [], []
    eslots = {}
    for q in range(8):
        m1, m2_ = {}, {}
        meta1q, meta2q = [], []
        for g in range(G):
            sel, vl = esel[q][g], vloc[q][g]
            u_src = src[g][sel]
            # ---- L1 ----
            offs, K, tiles = grids1[g]
            rk = rank1[q, g][vl]
            p_, col = _edge_slots(rk, offs)
            msg = np.zeros((P, K, C + 1), np.float32)
            msg[p_, col, 0:C] = label_init[u_src]
            msg[p_, col, C] = 1.0
            ee = np.full((P, K), -1e30, np.float32)
            ee[p_, col] = e_edge[0, g][sel]
            m1[f"msg{g}"] = msg.astype(bfloat16)
            m1[f"ee{g}"] = ee.astype(bfloat16)
            o1 = np.empty(SLAB, np.int64)
            o1[rank1[q, g]] = np.arange(SLAB)  # rank -> vloc
            ml_, moh_ = masks_for(o1, q)
            m1[f"ml{g}"] = ml_
            m1[f"moh{g}"] = moh_
            meta1q.append((K, tiles))
            # ---- L2 ----
            offs, K2, tiles2 = grids2[g]
            rk2 = rankT[q][vl]
            p2, col2 = _edge_slots(rk2, offs)
            eslots.setdefault(q, {})[g] = (sel, p2, col2, K2)
            ee2 = np.full((P, K2), -1e30, np.float32)
            ee2[p2, col2] = e_edge[1, g][sel]
            m2_[f"ee{g}"] = ee2.astype(bfloat16)
            mlT2, mohT2 = masks_for(orderT[q], q)
            m2_[f"ml{g}"] = mlT2
            m2_[f"moh{g}"] = mohT2
            meta2q.append((K2, tiles2))
        # final-stage tensors in rankT order
        vg = orderT[q] + S8 * q
        vgc = np.minimum(vg, N - 1)
        validn = (orderT[q] < S8).astype(np.float32)
        att = attention[vgc, :, 0] * validn[:, None]
        m2_["attf"] = att.reshape(ROWS, P, G).transpose(1, 0, 2).copy()
        m2_["alphaf"] = (alpha[vgc, 0] * validn).reshape(ROWS, P).T.copy()
        featv = features[vgc] * validn[:, None]
        m2_["featT"] = featv.T.reshape(4, P, SLAB).astype(bfloat16)
        m2_["w1"] = np.asarray(inputs["w1"]).astype(bfloat16)
        m2_["b1t"] = np.asarray(inputs["b1"]).astype(np.float32).reshape(H, 1)
        m2_["w2"] = np.asarray(inputs["w2"]).astype(np.float32)
        m2_["b2t"] = np.asarray(inputs["b2"]).astype(np.float32).reshape(C, 1)
        m2_["ident"] = np.eye(C, dtype=np.float32)
        in1.append(m1)
        in2.append(m2_)
        meta1.append(meta1q)
        meta2.append(meta2q)

    pr = {"in1": in1, "in2": in2, "meta1": meta1, "meta2": meta2,
          "rank1": rank1, "rankT": rankT, "orderT": orderT,
          "eslots": eslots}
    return pr


def _fill_l2_msgs(pr, h1full):
    """After launch 1: build layer-2 message payloads from assembled h1."""
    for q in range(8):
        m2_ = pr["in2"][q]
        for g in range(G):
            sel, p2, col2, K2 = pr["eslots"][q][g]
            src = pr["_src"]
            u_src = src[g][sel]
            msg = np.zeros((P, K2, C + 1), np.float32)
            msg[p2, col2, 0:C] = h1full[g][u_src]
            msg[p2, col2, C] = 1.0
            m2_[f"msg{g}"] = msg.astype(bfloat16)


def _patch_tile():
    import concourse.tile as tile
    import concourse.mybir as mybir
    from concourse.vector_clock import ScopedClock

    def _drain_and_barrier(self, tick_clock, wait_clock):
        nc = self.nc
        drain_inst = nc.sync.drain()
        wait_clock.add_sem_waits(
            drain_inst.ins, ScopedClock({None: tick_clock.global_clock}))
        si = drain_inst.ins.sync_info
        if si is not None and len(si.on_wait) > 1:
            waits = list(si.on_wait)
            si.on_wait = waits[:1]
            rest = waits[1:]
            while rest:
                extra = nc.sync.drain()
                chunk, rest = rest[:1], rest[1:]
                esi = extra.ins.sync_info
                if esi is None:
                    extra.ins.sync_info = mybir.SyncInfo(
                        on_wait=chunk, on_update=[])
                else:
                    esi.on_wait = chunk
        nc.all_engine_barrier()
        assert self.sems is not None
        popped = nc._tile_sem_poison_stack.pop()
        assert popped is self._sem_poison
        nc.clear_and_free_semaphores(list(self.sems.allocated().values()))
        nc.all_engine_barrier()

    tile.TileContext._drain_and_barrier = _drain_and_barrier


def _split_excess_waits(nc, limit=1):
    import concourse.mybir as mybir
    seen, bbs = set(), []
    for name, bbc in nc.bb_map.items():
        bb = bbc.bb if hasattr(bbc, "bb") else bbc
        if id(bb) not in seen:
            seen.add(id(bb))
            bbs.append(bb)
    cur = nc.cur_bb.bb
    for bb in bbs:
        insts = bb.instructions
        out, changed = [], False
        for inst in insts:
            si = inst.sync_info
            if si is not None and len(si.on_wait) > limit:
                waits = list(si.on_wait)
                keep, extra = waits[:limit], waits[limit:]
                for w in extra:
                    nop = nc.engines[inst.engine].nop().ins
                    cl = cur.instructions
                    assert cl and cl[-1].name == nop.name
                    cur.instructions = cl[:-1]
                    nop.sync_info = mybir.SyncInfo(on_wait=[w], on_update=[])
                    out.append(nop)
                si.on_wait = keep
                changed = True
            out.append(inst)
        if changed:
            bb.instructions = out



# ---------------------------------------------------------------------------
# device programs
# ---------------------------------------------------------------------------

def _layer_block(nc, tc, mb, pools, ext, meta_q, tag_sfx=""):
    """Emit per-graph edge-softmax + segment-sum for one launch.
    Returns list of h tiles (one per graph), each [P, ROWS, C] f32."""
    dt = mb.dt
    msgp, epool, accp = pools
    hs = []
    for g in range(G):
        K, tiles = meta_q[g]
        u = accp.tile([P, ROWS, C + 1], dt.float32, name=f"u{g}{tag_sfx}",
                      tag=f"u{g}")
        nc.vector.memset(u[:], 0.0)
        for t in range(K // CT):
            msg = msgp.tile([P, CT, C + 1], dt.bfloat16,
                            name=f"m{g}{t}{tag_sfx}", tag="msg")
            nc.sync.dma_start(
                out=msg[:], in_=ext[f"msg{g}"][:, t * CT:(t + 1) * CT, :])
            et = epool.tile([P, CT], dt.bfloat16, name=f"e{g}{t}{tag_sfx}",
                            tag="et")
            nc.sync.dma_start(out=et[:],
                              in_=ext[f"ee{g}"][:, t * CT:(t + 1) * CT])
            ex = epool.tile([P, CT], dt.bfloat16, name=f"x{g}{t}{tag_sfx}",
                            tag="ex")
            nc.scalar.activation(ex[:], et[:],
                                 mb.ActivationFunctionType.Exp)
            prod = msgp.tile([P, CT, C + 1], dt.float32,
                             name=f"p{g}{t}{tag_sfx}", tag="prod")
            nc.vector.tensor_tensor(
                out=prod[:], in0=msg[:],
                in1=ex[:].to_broadcast([P, CT, C + 1]),
                op=mb.AluOpType.mult)
            for (k0, nk, g_, off) in tiles[t]:
                inap = prod[:, off:off + nk * g_, :].rearrange(
                    "p (nk g) c -> p nk c g", g=g_)
                nc.vector.tensor_reduce(
                    out=u[:, k0:k0 + nk, :], in_=inap,
                    axis=mb.AxisListType.X, op=mb.AluOpType.add)
        ml = accp.tile([P, ROWS], dt.float32, name=f"ml{g}{tag_sfx}",
                       tag=f"ml{g}")
        nc.sync.dma_start(out=ml[:], in_=ext[f"ml{g}"][:])
        moh = accp.tile([P, ROWS, C], dt.float32, name=f"moh{g}{tag_sfx}",
                        tag=f"moh{g}")
        nc.sync.dma_start(out=moh[:], in_=ext[f"moh{g}"][:])
        s = u[:, :, C]
        nc.vector.tensor_scalar_max(s, s, 1.0)
        rec = accp.tile([P, ROWS], dt.float32, name=f"rc{g}{tag_sfx}",
                        tag=f"rec{g}")
        nc.vector.reciprocal(out=rec[:], in_=s)
        h = accp.tile([P, ROWS, C], dt.float32, name=f"h{g}{tag_sfx}",
                      tag=f"h{g}")
        nc.vector.tensor_tensor(
            out=h[:], in0=u[:, :, 0:C],
            in1=rec[:].to_broadcast([P, ROWS, C]), op=mb.AluOpType.mult)
        nc.vector.tensor_tensor(
            out=h[:], in0=h[:], in1=ml[:].to_broadcast([P, ROWS, C]),
            op=mb.AluOpType.mult)
        nc.vector.tensor_tensor(out=h[:], in0=h[:], in1=moh[:],
                                op=mb.AluOpType.add)
        hs.append(h)
    return hs


def _declare_layer_inputs(nc, dt, meta_q):
    ext = {}
    for g in range(G):
        K, _ = meta_q[g]
        ext[f"msg{g}"] = nc.declare_dram_parameter(
            f"msg{g}", [P, K, C + 1], dt.bfloat16, isOutput=False)
        ext[f"ee{g}"] = nc.declare_dram_parameter(
            f"ee{g}", [P, K], dt.bfloat16, isOutput=False)
        ext[f"ml{g}"] = nc.declare_dram_parameter(
            f"ml{g}", [P, ROWS], dt.float32, isOutput=False)
        ext[f"moh{g}"] = nc.declare_dram_parameter(
            f"moh{g}", [P, ROWS, C], dt.float32, isOutput=False)
    return ext


def _build_l1(meta_q):
    import concourse.bass as bass
    import concourse.mybir as mb
    from concourse.tile import TileContext

    _patch_tile()
    dt = mb.dt
    nc = bass.Bass("TRN2", target_bir_lowering=False, debug=False)
    ext = _declare_layer_inputs(nc, dt, meta_q)
    outs = [nc.declare_dram_parameter(f"out{g}", [P, ROWS, C], dt.float32,
                                      isOutput=True) for g in range(G)]
    with TileContext(nc) as tc:
        with (
            tc.tile_pool(name="msgp", bufs=3) as msgp,
            tc.tile_pool(name="epool", bufs=3) as epool,
            tc.tile_pool(name="accp", bufs=1) as accp,
        ):
            hs = _layer_block(nc, tc, mb, (msgp, epool, accp), ext, meta_q)
            for g in range(G):
                nc.sync.dma_start(out=outs[g][:], in_=hs[g][:])
    _split_excess_waits(nc)
    mb.codegen_inst_isa_subclasses(nc)
    return nc


def _build_l2(meta_q):
    import concourse.bass as bass
    import concourse.mybir as mb
    from concourse.tile import TileContext

    _patch_tile()
    dt = mb.dt
    nc = bass.Bass("TRN2", target_bir_lowering=False, debug=False)
    ext = _declare_layer_inputs(nc, dt, meta_q)
    bf16_params = {"featT", "w1"}
    for nm, shp in (("attf", [P, ROWS, G]), ("alphaf", [P, ROWS]),
                    ("featT", [4, P, SLAB]), ("w1", [F, H]),
                    ("b1t", [H, 1]), ("w2", [H, C]), ("b2t", [C, 1]),
                    ("ident", [C, C])):
        ext[nm] = nc.declare_dram_parameter(
            nm, shp, dt.bfloat16 if nm in bf16_params else dt.float32,
            isOutput=False)
    out_ext = nc.declare_dram_parameter("out", [P, ROWS, C], dt.float32,
                                        isOutput=True)
    with TileContext(nc) as tc:
        with (
            tc.tile_pool(name="msgp", bufs=3) as msgp,
            tc.tile_pool(name="epool", bufs=3) as epool,
            tc.tile_pool(name="accp", bufs=1) as accp,
            tc.tile_pool(name="wkf", bufs=2) as wkf,
            tc.tile_pool(name="psp", bufs=2, space="PSUM") as psp,
        ):
            hs = _layer_block(nc, tc, mb, (msgp, epool, accp), ext, meta_q)

            # attention softmax + logits
            att = accp.tile([P, ROWS, G], dt.float32, name="atts",
                            tag="atts")
            nc.sync.dma_start(out=att[:], in_=ext["attf"][:])
            ea = wkf.tile([P, ROWS, G], dt.float32, name="ea", tag="ea")
            nc.scalar.activation(ea[:], att[:],
                                 mb.ActivationFunctionType.Exp)
            easum = wkf.tile([P, ROWS], dt.float32, name="easum",
                             tag="easum")
            nc.vector.tensor_reduce(out=easum[:], in_=ea[:],
                                    axis=mb.AxisListType.X,
                                    op=mb.AluOpType.add)
            erec = wkf.tile([P, ROWS], dt.float32, name="erec", tag="easum")
            nc.vector.reciprocal(out=erec[:], in_=easum[:])
            logits = accp.tile([P, ROWS, C], dt.float32, name="logits",
                               tag="logits")
            t0 = wkf.tile([P, ROWS, C], dt.float32, name="t0", tag="t0")
            nc.vector.tensor_tensor(
                out=logits[:], in0=hs[0][:],
                in1=ea[:, :, 0].to_broadcast([P, ROWS, C]),
                op=mb.AluOpType.mult)
            nc.vector.tensor_tensor(
                out=t0[:], in0=hs[1][:],
                in1=ea[:, :, 1].to_broadcast([P, ROWS, C]),
                op=mb.AluOpType.mult)
            nc.vector.tensor_tensor(out=logits[:], in0=logits[:],
                                    in1=t0[:], op=mb.AluOpType.add)
            nc.vector.tensor_tensor(
                out=logits[:], in0=logits[:],
                in1=erec[:].to_broadcast([P, ROWS, C]),
                op=mb.AluOpType.mult)

            # MLP over slab nodes
            w1s = accp.tile([P, 4, H], dt.bfloat16, name="w1s",
                            tag="w1s")
            nc.sync.dma_start(out=w1s[:], in_=ext["w1"][:].rearrange(
                "(c p) h -> p c h", c=4))
            w2s = accp.tile([H, C], dt.float32, name="w2s", tag="w2s")
            nc.sync.dma_start(out=w2s[:], in_=ext["w2"][:])
            b1s = accp.tile([H, 1], dt.float32, name="b1s", tag="b1s")
            nc.sync.dma_start(out=b1s[:], in_=ext["b1t"][:])
            b2s = accp.tile([C, 1], dt.float32, name="b2s", tag="b2s")
            nc.sync.dma_start(out=b2s[:], in_=ext["b2t"][:])
            idn = accp.tile([C, C], dt.float32, name="idn", tag="idn")
            nc.sync.dma_start(out=idn[:], in_=ext["ident"][:])

            mlpn = accp.tile([P, ROWS, C], dt.float32, name="mlpn",
                             tag="mlpn")
            nblk = SLAB // MLPB  # 32 full blocks
            blocks = [(b * MLPB, MLPB) for b in range(nblk)]
            if SLAB % MLPB:
                blocks.append((nblk * MLPB, SLAB % MLPB))
            for bi, (c0, ncols) in enumerate(blocks):
                ps1 = psp.tile([H, ncols], dt.float32, name=f"ps1{bi}",
                               tag="ps1")
                for j in range(4):
                    xt = wkf.tile([P, ncols], dt.bfloat16,
                                  name=f"xt{bi}{j}", tag="xt")
                    nc.sync.dma_start(
                        out=xt[:], in_=ext["featT"][j, :, c0:c0 + ncols])
                    nc.tensor.matmul(out=ps1[:], lhsT=w1s[:, j, :],
                                     rhs=xt[:], start=(j == 0),
                                     stop=(j == 3))
                r1 = wkf.tile([H, ncols], dt.float32, name=f"r1{bi}",
                              tag="r1")
                nc.scalar.activation(r1[:], ps1[:],
                                     mb.ActivationFunctionType.Relu,
                                     bias=b1s[:])
                ps2 = psp.tile([C, ncols], dt.float32, name=f"ps2{bi}",
                               tag="ps2")
                nc.tensor.matmul(out=ps2[:], lhsT=w2s[:], rhs=r1[:],
                                 start=True, stop=True)
                m2 = wkf.tile([C, ncols], dt.float32, name=f"m2{bi}",
                              tag="m2")
                nc.vector.tensor_scalar_add(m2[:], ps2[:], b2s[:])
                for cch in range(ncols // P):
                    pst = psp.tile([P, C], dt.float32,
                                   name=f"pst{bi}{cch}", tag="pst")
                    nc.tensor.transpose(out=pst[:],
                                        in_=m2[:, cch * P:(cch + 1) * P],
                                        identity=idn[:])
                    nc.vector.tensor_copy(
                        out=mlpn[:, c0 // P + cch, :], in_=pst[:])

            alp = accp.tile([P, ROWS], dt.float32, name="alp", tag="alp")
            nc.sync.dma_start(out=alp[:], in_=ext["alphaf"][:])
            sgp = wkf.tile([P, ROWS], dt.float32, name="sgp", tag="sgp")
            nc.scalar.activation(sgp[:], alp[:],
                                 mb.ActivationFunctionType.Sigmoid)
            sgn = wkf.tile([P, ROWS], dt.float32, name="sgn", tag="sgn")
            nc.scalar.activation(sgn[:], alp[:],
                                 mb.ActivationFunctionType.Sigmoid,
                                 scale=-1.0)
            fout = accp.tile([P, ROWS, C], dt.float32, name="fout",
                             tag="fout")
            nc.vector.tensor_tensor(
                out=fout[:], in0=logits[:],
                in1=sgp[:].to_broadcast([P, ROWS, C]),
                op=mb.AluOpType.mult)
            t1 = wkf.tile([P, ROWS, C], dt.float32, name="t1", tag="t0")
            nc.vector.tensor_tensor(
                out=t1[:], in0=mlpn[:],
                in1=sgn[:].to_broadcast([P, ROWS, C]),
                op=mb.AluOpType.mult)
            nc.vector.tensor_tensor(out=fout[:], in0=fout[:], in1=t1[:],
                                    op=mb.AluOpType.add)
            nc.sync.dma_start(out=out_ext[:], in_=fout[:])
    _split_excess_waits(nc)
    mb.codegen_inst_isa_subclasses(nc)
    return nc


def _kernel_host(**inputs):
    """Exact reference semantics in numpy (f32)."""
    src = np.asarray(inputs["src"]); dst = np.asarray(inputs["dst"])
    e_edge = np.asarray(inputs["e_edge"], dtype=np.float32)
    label_init = np.asarray(inputs["label_init"], dtype=np.float32)
    labels_one_hot = np.asarray(inputs["labels_one_hot"], dtype=np.float32)
    alpha = np.asarray(inputs["alpha"], dtype=np.float32)
    attention = np.asarray(inputs["attention"], dtype=np.float32)
    w1 = np.asarray(inputs["w1"], dtype=np.float32)
    b1 = np.asarray(inputs["b1"], dtype=np.float32)
    w2 = np.asarray(inputs["w2"], dtype=np.float32)
    b2 = np.asarray(inputs["b2"], dtype=np.float32)
    train_mask = np.asarray(inputs["train_mask"])
    mask = train_mask.astype(np.float32)
    masked_label = 1.0 - mask
    masked_one_hot = labels_one_hot * mask
    h_list = []
    for g in range(G):
        h = label_init
        d = dst[g]; s_ = src[g]
        for l in range(L):
            e = e_edge[l, g]
            m = np.full(N, -np.inf, np.float32)
            np.maximum.at(m, d, e)
            ex = np.exp(e - m[d])
            ssum = np.zeros(N, np.float32)
            np.add.at(ssum, d, ex)
            a = ex / ssum[d]
            hn = np.zeros((N, C), np.float32)
            np.add.at(hn, d, h[s_] * a[:, None])
            h = hn * masked_label + masked_one_hot
        h_list.append(h)
    x = np.stack(h_list, axis=-1)                      # [N, C, G]
    att = attention[..., 0]                            # [N, G]
    att = att - att.max(axis=1, keepdims=True)
    ea = np.exp(att)
    attn = ea / ea.sum(axis=1, keepdims=True)
    logits = np.einsum("ncg,ng->nc", x, attn)
    mlp = np.maximum(features_mm(inputs, w1) + b1, 0.0) @ w2 + b2
    sa = 1.0 / (1.0 + np.exp(-alpha))
    return (sa * logits + (1.0 - sa) * mlp).astype(np.float32)


def features_mm(inputs, w1):
    f = np.asarray(inputs["features"], dtype=np.float32)
    return f @ w1



def kernel(**inputs):
    import os
    if os.environ.get("GNN_HOST") == "1":
        return _kernel_host(**inputs)
    import time
    from concourse.bass_utils import run_bass_kernel_spmd

    t0 = time.perf_counter()
    pr = _host_prep(inputs)
    pr["_src"] = np.asarray(inputs["src"])

    meta1 = pr["meta1"]
    meta2 = pr["meta2"]

    key1 = "l1" + str(meta1[0])
    if key1 not in _CACHE:
        _CACHE[key1] = _build_l1(meta1[0])
    nc1 = _CACHE[key1]
    t1 = time.perf_counter()
    res1 = run_bass_kernel_spmd(nc1, pr["in1"], list(range(8)))
    _CACHE["res1"] = res1
    t2 = time.perf_counter()

    h1full = [np.zeros((N, C), np.float32) for _ in range(G)]
    for q in range(8):
        for g in range(G):
            hq = res1.results[q][f"out{g}"]          # [P, ROWS, C] by rank
            nat = np.asarray(hq).transpose(1, 0, 2).reshape(SLAB, C)
            h1full[g][S8 * q:S8 * (q + 1)] = nat[pr["rank1"][q, g][:S8]]
    _fill_l2_msgs(pr, h1full)
    t3 = time.perf_counter()

    key2 = "l2" + str(meta2[0])
    if key2 not in _CACHE:
        _CACHE[key2] = _build_l2(meta2[0])
    nc2 = _CACHE[key2]
    t4 = time.perf_counter()
    res2 = run_bass_kernel_spmd(nc2, pr["in2"], list(range(8)))
    _CACHE["res"] = res2
    t5 = time.perf_counter()
    import sys
    print(f"[kernel] prep {t1-t0:.2f}s run1 {t2-t1:.2f}s fill {t3-t2:.2f}s "
          f"build2 {t4-t3:.2f}s run2 {t5-t4:.2f}s", file=sys.stderr)

    out = np.zeros((N, C), np.float32)
    for q in range(8):
        oq = np.asarray(res2.results[q]["out"]).transpose(
            1, 0, 2).reshape(SLAB, C)
        out[S8 * q:S8 * (q + 1)] = oq[pr["rankT"][q][:S8]]
    return out
